# revision 1
# baseline (speedup 1.0000x reference)
import sys

sys.path.insert(0, "/opt/trn_rl_repo")

import numpy as np
import ml_dtypes

# Phi3SeerAttention, B=1 S=2048 HIDDEN=3072, H=32 q heads, HK=8 kv heads,
# D=96, gate block 64, gate hidden 128. Sharded TP over kv heads: core c
# owns kv head c and q heads 4c..4c+3; o-proj row-sharded, partials summed
# on host (the gather step).
H, HK, D, BLK, GH = 32, 8, 96, 64, 128
S, HIDDEN = 2048, 3072
G = H // HK          # 4 q heads per kv head (per core)
NB = S // BLK        # 32 gate blocks
KT = HIDDEN // 128   # 24 contraction tiles
NS = S // 512        # 4 sequence chunks of 512
NT = S // 128        # 16 t-tiles of 128
NE = HIDDEN // 512   # 6 output column chunks
NCORES = 8
THR = 0.03

_prog = None


def _build(debug=False):
    from concourse import bass, mybir, bacc
    import concourse.tile as tile
    from contextlib import ExitStack

    dt = mybir.dt
    BF, F32 = dt.bfloat16, dt.float32
    AF = mybir.ActivationFunctionType
    OP = mybir.AluOpType
    AX = mybir.AxisListType.X

    nc = bacc.Bacc()
    xt_d = nc.dram_tensor("xt", [HIDDEN, S], BF, kind="ExternalInput")
    wq_d = nc.dram_tensor("wq", [HIDDEN, G * D], BF, kind="ExternalInput")
    wk_d = nc.dram_tensor("wk", [HIDDEN, D], BF, kind="ExternalInput")
    wv_d = nc.dram_tensor("wv", [HIDDEN, D], BF, kind="ExternalInput")
    ow_d = nc.dram_tensor("ow", [G * D, HIDDEN], BF, kind="ExternalInput")
    cosq_d = nc.dram_tensor("cosq", [D, S], BF, kind="ExternalInput")
    sinq_d = nc.dram_tensor("sinq", [D, S], BF, kind="ExternalInput")
    cosk_d = nc.dram_tensor("cosk", [D, S], BF, kind="ExternalInput")
    sink_d = nc.dram_tensor("sink", [D, S], BF, kind="ExternalInput")
    rot_d = nc.dram_tensor("rot", [D, D], BF, kind="ExternalInput")
    gwq_d = nc.dram_tensor("gwq", [D, GH], F32, kind="ExternalInput")
    gwk_d = nc.dram_tensor("gwk", [2 * D, GH], F32, kind="ExternalInput")
    eye_d = nc.dram_tensor("eye32", [NB, NB], F32, kind="ExternalInput")
    emat_d = nc.dram_tensor("emat", [NB, NT * 128], F32, kind="ExternalInput")
    bcm_d = nc.dram_tensor("bcm", [NB, NB], F32, kind="ExternalInput")
    cmask_d = nc.dram_tensor("cmask", [128, 4 * 512], BF, kind="ExternalInput")
    out_d = nc.dram_tensor("out_p", [S, HIDDEN], BF, kind="ExternalOutput")

    # Raw (persistent) SBUF tensors that cross the phase-1 barrier. The two
    # TileContexts are separated by a full drain+barrier so no instruction
    # ever needs to wait on the union of all 8 DMA HW queue semaphores
    # (compute-engine instructions have a small embedded sync-wait cap).
    q_sb = nc.alloc_sbuf_tensor("q_sbuf", [D, G, S], BF)
    k_sb = nc.alloc_sbuf_tensor("k_sbuf", [D, S], BF)
    v_sb = nc.alloc_sbuf_tensor("v_sbuf", [128, NT, D + 1], BF)
    qp_sb = nc.alloc_sbuf_tensor("qp_sbuf", [D, G, NB], F32)
    km_sb = nc.alloc_sbuf_tensor("km_sbuf", [D, NB], F32)
    kx_sb = nc.alloc_sbuf_tensor("kx_sbuf", [D, NB], F32)

    # ---- context A / phase 1: QKV projection + gate pooling ----
    with tile.TileContext(nc) as tc:
        with tc.tile_pool(name="xw", bufs=1) as xw, tc.tile_pool(
            name="ps1", bufs=6, space="PSUM"
        ) as ps1:
            xt_sb = xw.tile([128, KT, S], BF)
            wq_sb = xw.tile([128, KT, G * D], BF)
            wk_sb = xw.tile([128, KT, D], BF)
            wv_sb = xw.tile([128, KT, D], BF)
            for kt in range(KT):
                r = slice(kt * 128, (kt + 1) * 128)
                nc.sync.dma_start(wq_sb[:, kt, :], wq_d[r, :])
                nc.sync.dma_start(wk_sb[:, kt, :], wk_d[r, :])
                nc.sync.dma_start(wv_sb[:, kt, :], wv_d[r, :])
            nc.vector.memset(v_sb[:, :, D : D + 1], 1.0)

            for j in range(NS):
                sl = slice(j * 512, (j + 1) * 512)
                for kt in range(KT):
                    r = slice(kt * 128, (kt + 1) * 128)
                    nc.sync.dma_start(xt_sb[:, kt, sl], xt_d[r, sl])

                for hh in range(G + 1):  # 0..3 = q heads, 4 = k
                    ps = ps1.tile([128, 512], F32)
                    pq = ps[:D, :]
                    for kt in range(KT):
                        lhsT = (
                            wq_sb[:, kt, hh * D : (hh + 1) * D]
                            if hh < G
                            else wk_sb[:, kt, :]
                        )
                        nc.tensor.matmul(
                            pq,
                            lhsT,
                            xt_sb[:, kt, sl],
                            start=(kt == 0),
                            stop=(kt == KT - 1),
                        )
                    pr = pq.rearrange("p (b w) -> p b w", w=BLK)
                    bs = slice(j * 8, (j + 1) * 8)
                    if hh < G:
                        # block SUM; 1/BLK folded into gate scale
                        nc.vector.tensor_reduce(
                            qp_sb[:, hh, bs], pr, axis=AX, op=OP.add
                        )
                        nc.scalar.copy(q_sb[:, hh, sl], pq)
                    else:
                        # block SUM; 1/BLK folded into gwk rows on host
                        nc.vector.tensor_reduce(km_sb[:, bs], pr, axis=AX, op=OP.add)
                        nc.vector.tensor_reduce(kx_sb[:, bs], pr, axis=AX, op=OP.max)
                        nc.scalar.copy(k_sb[:, sl], pq)

                for ti in range(4 * j, 4 * (j + 1)):
                    ps = ps1.tile([128, 512], F32)
                    pv = ps[:, :D]
                    for kt in range(KT):
                        nc.tensor.matmul(
                            pv,
                            xt_sb[:, kt, ti * 128 : (ti + 1) * 128],
                            wv_sb[:, kt, :],
                            start=(kt == 0),
                            stop=(kt == KT - 1),
                        )
                    nc.scalar.copy(v_sb[:, ti, :D], pv)

    # ---- context B: gate, RoPE, attention, o-projection ----
    with tile.TileContext(nc) as tc:
        with ExitStack() as ctx:
            perm = ctx.enter_context(tc.tile_pool(name="perm", bufs=1))
            mask_sb = perm.tile([128, NT, NB], BF)
            rot_sb = perm.tile([D, D], BF)
            gwq_sb = perm.tile([D, GH], F32)
            gwk_sb = perm.tile([D, 2, GH], F32)
            eye_sb = perm.tile([NB, NB], F32)
            bcm_sb = perm.tile([NB, NB], F32)
            ones_sb = perm.tile([1, 128], BF)
            attn_sb = perm.tile([D, G, S], BF)  # normalized attn output^T
            cosq_sb = perm.tile([D, S], BF)
            sinq_sb = perm.tile([D, S], BF)
            cosk_sb = perm.tile([D, S], BF)
            sink_sb = perm.tile([D, S], BF)
            emat_sb = perm.tile([NB, NT * 128], F32)
            cmask_sb = perm.tile([128, 4 * 512], BF)
            ow_sb = perm.tile([D, G, HIDDEN], BF)

            nc.sync.dma_start(rot_sb[:], rot_d[:])
            nc.sync.dma_start(gwq_sb[:], gwq_d[:])
            nc.sync.dma_start(gwk_sb[:, 0, :], gwk_d[0:D, :])
            nc.sync.dma_start(gwk_sb[:, 1, :], gwk_d[D : 2 * D, :])
            nc.sync.dma_start(eye_sb[:], eye_d[:])
            nc.sync.dma_start(bcm_sb[:], bcm_d[:])
            nc.sync.dma_start(cosq_sb[:], cosq_d[:])
            nc.sync.dma_start(sinq_sb[:], sinq_d[:])
            nc.sync.dma_start(cosk_sb[:], cosk_d[:])
            nc.sync.dma_start(sink_sb[:], sink_d[:])
            nc.sync.dma_start(emat_sb[:], emat_d[:])
            nc.sync.dma_start(cmask_sb[:], cmask_d[:])
            for hh in range(G):
                nc.sync.dma_start(ow_sb[:, hh, :], ow_d[hh * D : (hh + 1) * D, :])
            nc.vector.memset(ones_sb[:], 1.0)

            # ---- phase 2: block gate (fp32) ----
            with tc.tile_pool(name="gp", bufs=1) as gp, tc.tile_pool(
                name="gps", bufs=1, space="PSUM"
            ) as gps, tc.tile_pool(name="gpsm", bufs=2, space="PSUM") as gpsm:
                t0 = gp.tile([D, NB], F32)
                qps = gp.tile([D, NB], F32)
                nc.vector.tensor_add(t0[:], qp_sb[:, 0, :], qp_sb[:, 1, :])
                nc.vector.tensor_add(qps[:], qp_sb[:, 2, :], qp_sb[:, 3, :])
                nc.vector.tensor_add(qps[:], t0[:], qps[:])

                kg_ps = gps.tile([NB, GH], F32)
                nc.tensor.matmul(kg_ps, km_sb[:], gwk_sb[:, 0, :], start=True, stop=False)
                nc.tensor.matmul(kg_ps, kx_sb[:], gwk_sb[:, 1, :], start=False, stop=True)
                qg_ps = gps.tile([NB, GH], F32)
                nc.tensor.matmul(qg_ps, qps[:], gwq_sb[:], start=True, stop=True)
                qg_sb = gp.tile([NB, GH], F32)
                kg_sb = gp.tile([NB, GH], F32)
                # fold mean-over-heads (1/G), block mean (1/BLK), GH^-0.5
                nc.scalar.mul(qg_sb[:], qg_ps[:], (1.0 / (G * BLK)) * GH**-0.5)
                nc.scalar.copy(kg_sb[:], kg_ps[:])

                qgT_ps = gps.tile([GH, NB], F32)
                nc.tensor.matmul(qgT_ps, qg_sb[:], eye_sb[:], start=True, stop=True)
                kgT_ps = gps.tile([GH, NB], F32)
                nc.tensor.matmul(kgT_ps, kg_sb[:], eye_sb[:], start=True, stop=True)
                qgT_sb = gp.tile([GH, NB], F32)
                kgT_sb = gp.tile([GH, NB], F32)
                nc.scalar.copy(qgT_sb[:], qgT_ps[:])
                nc.scalar.copy(kgT_sb[:], kgT_ps[:])

                lg_ps = gps.tile([NB, NB], F32)
                nc.tensor.matmul(lg_ps, qgT_sb[:], kgT_sb[:], start=True, stop=True)
                lg_sb = gp.tile([NB, NB], F32)
                nc.scalar.copy(lg_sb[:], lg_ps[:])
                lm_sb = gp.tile([NB, NB], F32)
                nc.vector.tensor_add(lm_sb[:], lg_sb[:], bcm_sb[:])
                ge_sb = gp.tile([NB, NB], F32)
                gsum = gp.tile([NB, 1], F32)
                nc.scalar.activation(ge_sb[:], lm_sb[:], AF.Exp, accum_out=gsum[:])
                grc = gp.tile([NB, 1], F32)
                nc.vector.reciprocal(grc[:], gsum[:])
                prob_sb = gp.tile([NB, NB], F32)
                nc.scalar.activation(prob_sb[:], ge_sb[:], AF.Copy, scale=grc[:])
                m01 = gp.tile([NB, NB], F32)
                nc.vector.tensor_scalar(m01[:], prob_sb[:], THR, None, op0=OP.is_ge)
                nc.vector.tensor_tensor(m01[:], m01[:], eye_sb[:], op=OP.max)
                # transpose: expansion partitions index k blocks, m01 rows
                # index q blocks
                m01t_ps = gps.tile([NB, NB], F32)
                nc.tensor.matmul(m01t_ps, m01[:], eye_sb[:], start=True, stop=True)
                m01t = gp.tile([NB, NB], F32)
                nc.scalar.copy(m01t[:], m01t_ps[:])

                if debug:
                    for nm, t in [
                        ("dlg", lg_sb),
                        ("dqg", qg_sb),
                        ("dkg", kg_sb),
                        ("dprob", prob_sb),
                        ("dm01", m01),
                    ]:
                        dd = nc.dram_tensor(
                            nm, list(t[:].shape), t[:].dtype, kind="ExternalOutput"
                        )
                        nc.sync.dma_start(dd[:], t[:])

                for i in range(NT):
                    mp = gpsm.tile([128, NB], F32)
                    nc.tensor.matmul(
                        mp,
                        emat_sb[:, i * 128 : (i + 1) * 128],
                        m01t[:],
                        start=True,
                        stop=True,
                    )
                    nc.scalar.copy(mask_sb[:, i, :], mp[:])

            # ---- phase 3: RoPE in place on q^T / k^T ----
            with tc.tile_pool(name="rp", bufs=4) as rp, tc.tile_pool(
                name="rps", bufs=4, space="PSUM"
            ) as rps:
                for hh in range(G + 1):
                    src = q_sb[:, hh, :] if hh < G else k_sb[:]
                    cs = cosq_sb if hh < G else cosk_sb
                    sn = sinq_sb if hh < G else sink_sb
                    for j in range(NS):
                        sl = slice(j * 512, (j + 1) * 512)
                        rt = rps.tile([D, 512], F32)
                        nc.tensor.matmul(rt, rot_sb[:], src[:, sl], start=True, stop=True)
                        t1 = rp.tile([D, 512], BF)
                        nc.vector.tensor_mul(t1[:], src[:, sl], cs[:, sl])
                        t2 = rp.tile([D, 512], BF)
                        nc.vector.tensor_mul(t2[:], rt[:], sn[:, sl])
                        nc.vector.tensor_add(src[:, sl], t1[:], t2[:])

            # ---- phase 4: masked attention (transposed P layout) ----
            from concourse.bass import AP

            with tc.tile_pool(name="ap_", bufs=4) as ap_, tc.tile_pool(
                name="sm", bufs=4
            ) as sm, tc.tile_pool(name="sps", bufs=3, space="PSUM") as sps, tc.tile_pool(
                name="pvs", bufs=2, space="PSUM"
            ) as pvs, tc.tile_pool(name="rbs", bufs=2, space="PSUM") as rbs:
                for hh in range(G):
                    for j in range(NS):
                        ssl = slice(j * 512, (j + 1) * 512)
                        pv_ps = pvs.tile([D + 1, 512], F32)
                        ntile = 4 * (j + 1)
                        for ti in range(ntile):
                            s_ps = sps.tile([128, 512], F32)
                            nc.tensor.matmul(
                                s_ps,
                                k_sb[:, ti * 128 : (ti + 1) * 128],
                                q_sb[:, hh, ssl],
                                start=True,
                                stop=True,
                                skip_group_check=True,
                            )
                            p_sb = ap_.tile([128, 512], BF)
                            nc.scalar.activation(p_sb[:], s_ps[:], AF.Exp)
                            if ti >= 4 * j:
                                r = ti - 4 * j
                                nc.vector.tensor_mul(
                                    p_sb[:],
                                    p_sb[:],
                                    cmask_sb[:, r * 512 : (r + 1) * 512],
                                )
                            msl = mask_sb[:, ti, j * 8 : (j + 1) * 8]
                            mb = AP(
                                tensor=msl.tensor,
                                offset=msl.offset,
                                ap=list(msl.ap) + [[0, BLK]],
                            )
                            p3 = p_sb[:].rearrange("p (b w) -> p b w", w=BLK)
                            nc.vector.tensor_tensor(p3, p3, mb, op=OP.mult)
                            nc.tensor.matmul(
                                pv_ps,
                                v_sb[:, ti, :],
                                p_sb[:],
                                start=(ti == 0),
                                stop=(ti == ntile - 1),
                                skip_group_check=True,
                            )
                        sr = sm.tile([1, 512], F32)
                        nc.scalar.copy(sr[:], pv_ps[D : D + 1, :])
                        rc = sm.tile([1, 512], F32)
                        nc.vector.reciprocal(rc[:], sr[:])
                        rcb = sm.tile([1, 512], BF)
                        nc.vector.tensor_copy(rcb[:], rc[:])
                        rb_ps = rbs.tile([D, 512], F32)
                        nc.tensor.matmul(
                            rb_ps, ones_sb[:, :D], rcb[:], start=True, stop=True
                        )
                        # HW: DVE may read only ONE input from PSUM
                        rb_sb = sm.tile([D, 512], F32)
                        nc.scalar.copy(rb_sb[:], rb_ps[:])
                        nc.vector.tensor_mul(
                            attn_sb[:, hh, ssl], pv_ps[:D, :], rb_sb[:]
                        )

            # ---- phase 5: o-projection partial ----
            with tc.tile_pool(name="op_", bufs=4) as op_, tc.tile_pool(
                name="ops", bufs=4, space="PSUM"
            ) as ops:
                for si in range(NT):
                    tsl = slice(si * 128, (si + 1) * 128)
                    for ej in range(NE):
                        esl = slice(ej * 512, (ej + 1) * 512)
                        o_ps = ops.tile([128, 512], F32)
                        for hh in range(G):
                            nc.tensor.matmul(
                                o_ps,
                                attn_sb[:, hh, tsl],
                                ow_sb[:, hh, esl],
                                start=(hh == 0),
                                stop=(hh == G - 1),
                            )
                        o_sb = op_.tile([128, 512], BF)
                        nc.scalar.copy(o_sb[:], o_ps[:])
                        nc.sync.dma_start(out_d[tsl, esl], o_sb[:])

            if debug:
                for nm, t in [
                    ("dq", q_sb),
                    ("dk", k_sb),
                    ("dv", v_sb),
                    ("dmask", mask_sb),
                    ("dqp", qp_sb),
                    ("dkm", km_sb),
                    ("dkx", kx_sb),
                    ("dattn", attn_sb),
                ]:
                    dd = nc.dram_tensor(
                        nm, list(t[:].shape), t[:].dtype, kind="ExternalOutput"
                    )
                    nc.sync.dma_start(dd[:], t[:])
    return nc


def _host_prep(hidden_states, cos, sin, qkv_w, o_w, gate_wq, gate_wk):
    bf = ml_dtypes.bfloat16
    X = np.asarray(hidden_states, np.float32).reshape(S, HIDDEN)
    qkv_w = np.asarray(qkv_w, np.float32)
    o_w = np.asarray(o_w, np.float32)
    cos = np.asarray(cos, np.float32)
    sin = np.asarray(sin, np.float32)

    xt = np.ascontiguousarray(X.T).astype(bf)
    scale = D**-0.5
    cosT = np.ascontiguousarray(cos.T)
    sinT = np.ascontiguousarray(sin.T)
    cosq = (cosT * scale).astype(bf)
    sinq = (sinT * scale).astype(bf)
    cosk = cosT.astype(bf)
    sink = sinT.astype(bf)

    rt = np.zeros((D, D), np.float32)
    h = D // 2
    rt[np.arange(h) + h, np.arange(h)] = -1.0
    rt[np.arange(h), np.arange(h) + h] = 1.0
    rt = rt.astype(bf)

    emat = np.zeros((NB, NT * 128), np.float32)
    for i in range(NT):
        for p in range(128):
            emat[2 * i + p // BLK, i * 128 + p] = 1.0
    eye = np.eye(NB, dtype=np.float32)

    bcm = np.where(
        np.arange(NB)[None, :] <= np.arange(NB)[:, None], 0.0, -60.0
    ).astype(np.float32)
    # cmask[p, r*512+col] = 1 if col - p >= 128*r (k token ti*128+p causal
    # w.r.t. q token j*512+col on diagonal tiles, r = ti - 4j)
    p_i = np.arange(128)[:, None]
    cmask = np.zeros((128, 4 * 512), np.float32)
    for r in range(4):
        col = np.arange(512)[None, :]
        cmask[:, r * 512 : (r + 1) * 512] = (col - p_i >= 128 * r).astype(
            np.float32
        )
    cmask = cmask.astype(bf)

    # k block mean is computed on-device as a SUM; fold 1/BLK into the
    # mean-pool half of gate_wk
    gwk_s = np.asarray(gate_wk, np.float32).copy()
    gwk_s[:D, :] *= 1.0 / BLK

    common = dict(
        xt=xt,
        cosq=cosq,
        sinq=sinq,
        cosk=cosk,
        sink=sink,
        rot=rt,
        gwq=np.asarray(gate_wq, np.float32),
        gwk=gwk_s,
        eye32=eye,
        emat=emat,
        bcm=bcm,
        cmask=cmask,
    )
    maps = []
    for c in range(NCORES):
        maps.append(
            dict(
                common,
                wq=qkv_w[:, c * G * D : (c + 1) * G * D].astype(bf),
                wk=qkv_w[:, H * D + c * D : H * D + (c + 1) * D].astype(bf),
                wv=qkv_w[
                    :, H * D + HK * D + c * D : H * D + HK * D + (c + 1) * D
                ].astype(bf),
                ow=o_w[c * G * D : (c + 1) * G * D, :].astype(bf),
            )
        )
    return maps


def _gather(results):
    acc = np.zeros((S, HIDDEN), np.float32)
    for r in results:
        acc += np.asarray(r["out_p"]).astype(np.float32)
    return acc.reshape(1, S, HIDDEN)


def _run(inputs, trace=False):
    global _prog
    if _prog is None:
        _prog = _build()
        if not _prog.is_finalized():
            _prog.finalize()
    from concourse import bass_utils

    maps = _host_prep(**inputs)
    res = bass_utils.run_bass_kernel_spmd(
        _prog, maps, list(range(NCORES)), trace=trace
    )
    return _gather(res.results), res


def kernel(**inputs):
    out, _ = _run(inputs, trace=False)
    return out



# revision 34
# speedup vs baseline: 1.7035x; 1.7035x over previous
import sys

sys.path.insert(0, "/opt/trn_rl_repo")

import numpy as np
import ml_dtypes

# Phi3SeerAttention, B=1 S=2048 HIDDEN=3072, H=32 q heads, HK=8 kv heads,
# D=96, gate block 64, gate hidden 128. Sharded TP over kv heads: core c
# owns kv head c and q heads 4c..4c+3; o-proj row-sharded, partials summed
# on host (the gather step).
#
# Single fused per-chunk pipeline (chunk = 512 tokens):
#   QKV (packed 128-row weight tiles) -> gate-row -> RoPE -> attention
#   -> o-proj(prev chunk), so PE never drains between phases.
H, HK, D, BLK, GH = 32, 8, 96, 64, 128
S, HIDDEN = 2048, 3072
G = H // HK          # 4 q heads per kv head (per core)
NB = S // BLK        # 32 gate blocks
KT = HIDDEN // 128   # 24 contraction tiles
NS = S // 512        # 4 sequence chunks of 512
NT = S // 128        # 16 t-tiles of 128
NE = HIDDEN // 512   # 6 output column chunks
QK = G * D + D       # 480 packed q+k output dims (q = rows 0..383, k = 384..479)
NCORES = 8
THR = 0.03

_prog = None


def _build(debug=False):
    from concourse import bass, mybir, bacc
    from concourse.bass import AP
    import concourse.tile as tile
    from contextlib import ExitStack

    dt = mybir.dt
    BF, F32 = dt.bfloat16, dt.float32
    AF = mybir.ActivationFunctionType
    OP = mybir.AluOpType
    AX = mybir.AxisListType.X

    nc = bacc.Bacc()
    xt_d = nc.dram_tensor("xt", [HIDDEN, S], BF, kind="ExternalInput")
    wqk_d = nc.dram_tensor("wqk", [HIDDEN, QK], BF, kind="ExternalInput")
    wv_d = nc.dram_tensor("wv", [HIDDEN, D], BF, kind="ExternalInput")
    owp_d = nc.dram_tensor("owp", [128, 3 * HIDDEN], BF, kind="ExternalInput")
    cosq_d = nc.dram_tensor("cosq", [D, S], BF, kind="ExternalInput")
    sinq_d = nc.dram_tensor("sinq", [D, S], BF, kind="ExternalInput")
    cosk_d = nc.dram_tensor("cosk", [D, S], BF, kind="ExternalInput")
    sink_d = nc.dram_tensor("sink", [D, S], BF, kind="ExternalInput")
    gwqp_d = nc.dram_tensor("gwqp", [128, 3 * GH], F32, kind="ExternalInput")
    gwk_d = nc.dram_tensor("gwk", [2 * D, GH], F32, kind="ExternalInput")
    eye8_d = nc.dram_tensor("eye8", [8, 8], F32, kind="ExternalInput")
    eyer_d = nc.dram_tensor("eyer", [8, NS * NB], F32, kind="ExternalInput")
    emat_d = nc.dram_tensor("emat", [NB, NT * 128], BF, kind="ExternalInput")
    bcm_d = nc.dram_tensor("bcm", [8, NS * NB], F32, kind="ExternalInput")
    cmask_d = nc.dram_tensor("cmask", [128, 512], BF, kind="ExternalInput")
    out_d = nc.dram_tensor("out_p", [S, HIDDEN], BF, kind="ExternalOutput")

    # de-interleave map: packed q row r = 96*h + d lives in tile r//128,
    # partition r%128.  pieces[h] = [(tile, psrc0, dsrc0, len), ...]
    qpieces = {
        0: [(0, 0, 0, 96)],
        1: [(0, 96, 0, 32), (1, 0, 32, 64)],
        2: [(1, 64, 0, 64), (2, 0, 64, 32)],
        3: [(2, 32, 0, 96)],
    }

    with tile.TileContext(nc) as tc:
        with ExitStack() as ctx:
            perm = ctx.enter_context(tc.tile_pool(name="perm", bufs=1))
            # weights / tables
            wqk_sb = perm.tile([128, KT, QK], BF)
            wv_sb = perm.tile([128, KT, D], BF)
            owp_sb = perm.tile([128, 3, HIDDEN], BF)
            gwqp_sb = perm.tile([128, 3, GH], F32)
            gwk_sb = perm.tile([D, 2, GH], F32)
            shuf_sb = perm.tile([D, G + 1, 512], BF)
            cosq_sb = perm.tile([D, S], BF)
            sinq_sb = perm.tile([D, S], BF)
            cosk_sb = perm.tile([D, S], BF)
            sink_sb = perm.tile([D, S], BF)
            eye8_sb = perm.tile([8, 8], F32)
            eyer_sb = perm.tile([8, NS, NB], F32)
            emat_sb = perm.tile([NB, NT * 128], BF)
            bcm_sb = perm.tile([8, NS, NB], F32)
            cmask_sb = perm.tile([128, 512], BF)
            ones_sb = perm.tile([1, 128], BF)
            # activations
            q_sb = perm.tile([D, G, S], BF)
            k_sb = perm.tile([D, S], BF)
            v_sb = perm.tile([128, NT, D + 1], BF)
            qkp_sb = perm.tile([128, 3, 512], BF)   # packed q of current chunk
            qp_sb = perm.tile([128, 3, NB], F32)    # packed q block-sums
            km_sb = perm.tile([D, NB], F32)
            kx_sb = perm.tile([D, NB], F32)
            qgT_sb = perm.tile([GH, NB], F32)
            kgT_sb = perm.tile([GH, NB], F32)
            mask_sb = perm.tile([128, NT, NB], BF)
            attn_sb = perm.tile([D, G, S], BF)
            attnp_sb = perm.tile([128, 3, S], BF)
            xt_sb = perm.tile([128, 2, KT, 512], BF)  # double-buffered x^T

            # priority order: the first QKV matmuls need xt chunk0 + wqk;
            # interleave 3-kt batches of both so PE can start ~2.5us in.
            groups = [1, 2, 3, 3, 3, 3, 3, 3, 3]
            k0 = 0
            for kb in groups:
                ks = slice(k0 * 128, (k0 + kb) * 128)
                nc.sync.dma_start(
                    xt_sb[:, 0, k0 : k0 + kb, :],
                    xt_d[ks, 0:512].rearrange("(k p) c -> p k c", p=128),
                )
                nc.gpsimd.dma_start(
                    wqk_sb[:, k0 : k0 + kb, :],
                    wqk_d[ks, :].rearrange("(k p) c -> p k c", p=128),
                )
                k0 += kb
            nc.sync.dma_start(
                wv_sb[:], wv_d[:].rearrange("(k p) c -> p k c", p=128)
            )
            # tables go on the scalar queue: the Activation engine is idle
            # until the first exp and its DMA triggers don't contend with
            # the xt/wqk streams on sync/gpsimd
            nc.scalar.dma_start(cosq_sb[:], cosq_d[:])
            nc.scalar.dma_start(sinq_sb[:], sinq_d[:])
            nc.scalar.dma_start(cosk_sb[:], cosk_d[:])
            nc.scalar.dma_start(sink_sb[:], sink_d[:])
            for t in range(3):
                nc.scalar.dma_start(
                    gwqp_sb[:, t, :], gwqp_d[:, t * GH : (t + 1) * GH]
                )
            nc.scalar.dma_start(gwk_sb[:, 0, :], gwk_d[0:D, :])
            nc.scalar.dma_start(gwk_sb[:, 1, :], gwk_d[D : 2 * D, :])
            nc.scalar.dma_start(eye8_sb[:], eye8_d[:])
            nc.scalar.dma_start(eyer_sb[:], eyer_d[:])
            nc.scalar.dma_start(emat_sb[:], emat_d[:])
            nc.scalar.dma_start(bcm_sb[:], bcm_d[:])
            nc.scalar.dma_start(cmask_sb[:], cmask_d[:])
            nc.vector.memset(ones_sb[:], 1.0)
            nc.vector.memset(v_sb[:, :, D : D + 1], 1.0)

            psR = ctx.enter_context(
                tc.tile_pool(name="psR", bufs=6, space="PSUM")
            )
            psPV = ctx.enter_context(
                tc.tile_pool(name="psPV", bufs=2, space="PSUM")
            )
            psD = psR

            def oproj_tile(si):
                tsl = slice(si * 128, (si + 1) * 128)
                o_sb = osb.tile([128, NE, 512], BF)
                tail = si >= 4 * (NS - 1)
                engs = [nc.sync, nc.gpsimd, nc.scalar]
                if tail:
                    eng = engs[si % 3]
                    eng2 = engs[(si + 1) % 3]
                else:
                    eng = eng2 = nc.sync if si % 2 == 0 else nc.gpsimd
                oview = out_d[tsl, :].rearrange("p (e c) -> p e c", c=512)
                for ej in range(NE):
                    esl = slice(ej * 512, (ej + 1) * 512)
                    o_ps = psR.tile([128, 512], F32, tag="ps")
                    for t in range(3):
                        nc.tensor.matmul(
                            o_ps,
                            attnp_sb[:, t, tsl],
                            owp_sb[:, t, esl],
                            start=(t == 0),
                            stop=(t == 2),
                        )
                    if (si + ej) % 2 == 0:
                        nc.scalar.copy(o_sb[:, ej, :], o_ps[:])
                    else:
                        nc.vector.tensor_copy(o_sb[:, ej, :], o_ps[:])
                    if ej == 2:
                        eng.dma_start(oview[:, 0:3, :], o_sb[:, 0:3, :])
                eng2.dma_start(oview[:, 3:6, :], o_sb[:, 3:6, :])

            osb = ctx.enter_context(tc.tile_pool(name="osb", bufs=2))
            rope_sb = ctx.enter_context(tc.tile_pool(name="rope", bufs=2))
            pfull = ctx.enter_context(tc.tile_pool(name="pfull", bufs=5))
            gsb = ctx.enter_context(tc.tile_pool(name="gsb", bufs=1))
            nsb = ctx.enter_context(tc.tile_pool(name="nsb", bufs=2))

            deferred = []
            qkv_state = {}

            for j in range(NS):
                sl = slice(j * 512, (j + 1) * 512)
                bs = slice(j * 8, (j + 1) * 8)
                xb = j % 2
                w = 8 * (j + 1)  # gate row width in blocks

                if j == 1:
                    # o-proj weights: first consumer is oproj(0) below
                    for t in range(3):
                        nc.scalar.dma_start(
                            owp_sb[:, t, :], owp_d[:, t * HIDDEN : (t + 1) * HIDDEN]
                        )

                # ---- QKV: 3 packed q tiles + 1 k tile ----
                def v_tile(ti):
                    ps = psR.tile([128, 512], F32, tag="ps")
                    pv = ps[:, :D]
                    vxb = (ti // 4) % 2
                    for kt in range(KT):
                        nc.tensor.matmul(
                            pv,
                            xt_sb[:, vxb, kt, (ti % 4) * 128 : (ti % 4 + 1) * 128],
                            wv_sb[:, kt, :],
                            start=(kt == 0),
                            stop=(kt == KT - 1),
                        )
                    nc.scalar.copy(v_sb[:, ti, :D], pv)

                def qkv_tile(jj, t):
                    # one packed QKV weight tile (t=0..2: q, t=3: k) of
                    # chunk jj: matmuls + block-sum reduces; copies to SBUF
                    # happen in the gate section (or inline when pipelined)
                    jbs = slice(jj * 8, (jj + 1) * 8)
                    jxb = jj % 2
                    ps = psR.tile([128, 512], F32, tag="ps")
                    if t < 3:
                        for kt in range(KT):
                            nc.tensor.matmul(
                                ps,
                                wqk_sb[:, kt, t * 128 : (t + 1) * 128],
                                xt_sb[:, jxb, kt, :],
                                start=(kt == 0),
                                stop=(kt == KT - 1),
                            )
                        pr = ps[:].rearrange("p (b w) -> p b w", w=BLK)
                        nc.vector.tensor_reduce(
                            qp_sb[:, t, jbs], pr, axis=AX, op=OP.add
                        )
                        return ps
                    pk = ps[:D, :]
                    for kt in range(KT):
                        nc.tensor.matmul(
                            pk,
                            wqk_sb[:, kt, 384:480],
                            xt_sb[:, jxb, kt, :],
                            start=(kt == 0),
                            stop=(kt == KT - 1),
                        )
                    prk = pk.rearrange("p (b w) -> p b w", w=BLK)
                    nc.vector.tensor_reduce(km_sb[:, jbs], prk, axis=AX, op=OP.add)
                    nc.vector.tensor_reduce(kx_sb[:, jbs], prk, axis=AX, op=OP.max)
                    return pk

                if j in qkv_state:
                    qtiles, pk = qkv_state.pop(j)
                else:
                    qtiles = [qkv_tile(j, t) for t in range(3)]
                    pk = qkv_tile(j, 3)

                # ---- gate row (part A): feature matmuls; the chain-critical
                # ACT copies go first in the ACT queue, psum->SBUF copies of
                # the q/k tiles follow ----
                qg_ps = psD.tile([GH, 8], F32, tag="ps")
                for t in range(3):
                    nc.tensor.matmul(
                        qg_ps,
                        gwqp_sb[:, t, :],
                        qp_sb[:, t, bs],
                        start=(t == 0),
                        stop=(t == 2),
                    )
                nc.scalar.mul(qgT_sb[:, bs], qg_ps[:], (1.0 / (G * BLK)) * GH**-0.5)
                # v tiles + the deferred h3 o-proj/norm are pure-PE filler
                # that hides the gate's ACT/DVE chain from the in-order PE
                vlist = list(range(4 * j, 4 * (j + 1)))
                if j == 0:
                    vlist = [0, 1, 2, 3]
                elif j == 1:
                    vlist = [4, 5, 6]  # v7 fills an attn(0) bubble instead
                v_tile(vlist[0])
                v_tile(vlist[1])
                kg_ps = psD.tile([GH, 8], F32, tag="ps")
                nc.tensor.matmul(kg_ps, gwk_sb[:, 0, :], km_sb[:, bs], start=True, stop=False)
                nc.tensor.matmul(kg_ps, gwk_sb[:, 1, :], kx_sb[:, bs], start=False, stop=True)
                nc.scalar.copy(kgT_sb[:, bs], kg_ps[:])
                v_tile(vlist[2])
                lg_ps = psD.tile([8, NB], F32, tag="ps")
                nc.tensor.matmul(
                    lg_ps[:, :w], qgT_sb[:, bs], kgT_sb[:, :w], start=True, stop=True
                )
                lm = gsb.tile([8, NB], F32)
                nc.vector.tensor_add(lm[:, :w], lg_ps[:, :w], bcm_sb[:, j, :w])
                ge = gsb.tile([8, NB], F32)
                gsum = gsb.tile([8, 1], F32)
                nc.scalar.activation(ge[:, :w], lm[:, :w], AF.Exp, accum_out=gsum[:])
                grc = gsb.tile([8, 1], F32)
                nc.vector.reciprocal(grc[:], gsum[:])
                prob = gsb.tile([8, NB], F32)
                nc.scalar.activation(prob[:, :w], ge[:, :w], AF.Copy, scale=grc[:])
                m01 = gsb.tile([8, NB], F32)
                nc.vector.tensor_scalar(m01[:, :w], prob[:, :w], THR, None, op0=OP.is_ge)
                nc.vector.tensor_tensor(m01[:, :w], m01[:, :w], eyer_sb[:, j, :w], op=OP.max)
                for ti in vlist[3:]:
                    v_tile(ti)
                for fn in deferred:
                    fn()
                deferred.clear()
                if j == 1:
                    oproj_tile(0)

                m01t_ps = psD.tile([NB, 8], F32, tag="ps")
                nc.tensor.matmul(
                    m01t_ps[:w, :], m01[:, :w], eye8_sb[:], start=True, stop=True
                )
                m01t = gsb.tile([NB, 8], BF)
                nc.scalar.copy(m01t[:w, :], m01t_ps[:w, :])
                mp_ps = psD.tile([128, 4 * 8 * NS], F32, tag="ps")
                ntile = 4 * (j + 1)
                for ti in range(ntile):
                    nc.tensor.matmul(
                        mp_ps[:, ti * 8 : (ti + 1) * 8],
                        emat_sb[:w, ti * 128 : (ti + 1) * 128],
                        m01t[:w, :],
                        start=True,
                        stop=True,
                    )
                mpr = mp_ps[:, : ntile * 8].rearrange("p (t b) -> p t b", b=8)
                nc.scalar.copy(mask_sb[:, :ntile, bs], mpr)
                nc.scalar.copy(qkp_sb[:, 0, :], qtiles[0][:])
                nc.vector.tensor_copy(qkp_sb[:, 1, :], qtiles[1][:])
                nc.scalar.copy(qkp_sb[:, 2, :], qtiles[2][:])
                nc.vector.tensor_copy(k_sb[:, sl], pk)

                # de-interleave packed q -> per-head q_sb (partition shifts
                # need DMA)
                for hh in range(G):
                    for (t, ps0, pd0, ln) in qpieces[hh]:
                        nc.sync.dma_start(
                            q_sb[pd0 : pd0 + ln, hh, sl],
                            qkp_sb[ps0 : ps0 + ln, t, :],
                        )


                # prefetch next x^T chunk (batched, split across queues)
                if j + 1 < NS:
                    nsl = slice((j + 1) * 512, (j + 2) * 512)
                    for g in range(4):
                        ks = slice(g * 6 * 128, (g + 1) * 6 * 128)
                        eng = nc.sync if g % 2 == 0 else nc.gpsimd
                        eng.dma_start(
                            xt_sb[:, (j + 1) % 2, g * 6 : (g + 1) * 6, :],
                            xt_d[ks, nsl].rearrange("(k p) c -> p k c", p=128),
                        )

                # ---- RoPE in place on q^T / k^T of this chunk ----
                # rotate-half is a fixed +-48-partition permutation: build it
                # with two small SBUF->SBUF DMAs (sign is folded into the sin
                # tables on host), keeping the whole combine in 4x DVE mode.
                h2 = D // 2
                for hh in range(G + 1):
                    src = q_sb[:, hh, sl] if hh < G else k_sb[:, sl]
                    nc.sync.dma_start(shuf_sb[0:h2, hh, :], src[h2:D, :])
                    nc.sync.dma_start(shuf_sb[h2:D, hh, :], src[0:h2, :])
                for hh in range(G + 1):
                    src = q_sb[:, hh, sl] if hh < G else k_sb[:, sl]
                    cs = cosq_sb if hh < G else cosk_sb
                    sn = sinq_sb if hh < G else sink_sb
                    t1 = rope_sb.tile([D, 512], BF)
                    nc.vector.tensor_mul(t1[:], src, cs[:, sl])
                    t2 = rope_sb.tile([D, 512], BF)
                    nc.vector.tensor_mul(t2[:], shuf_sb[:, hh, :], sn[:, sl])
                    nc.vector.tensor_add(src, t1[:], t2[:])

                # ---- attention for this chunk ----
                # scores/exp/mask run PIPE tiles ahead of the PV accumulation
                # so the in-order PE queue never waits on the exp->mask chain;
                # o-proj tiles of the previous chunk are interleaved per head
                # to fill remaining PE bubbles.
                ntile = 4 * (j + 1)
                PIPE = 4
                head_state = {}

                def emit_score(hh, ti):
                        r = ti - 4 * j
                        if r < 0:
                            s_ps = psR.tile([128, 512], F32, tag="ps")
                            nc.tensor.matmul(
                                s_ps,
                                k_sb[:, ti * 128 : (ti + 1) * 128],
                                q_sb[:, hh, sl],
                                start=True,
                                stop=True,
                                skip_group_check=True,
                            )
                            p_sb = pfull.tile([128, 512], BF)
                            nc.scalar.activation(p_sb[:], s_ps[:], AF.Exp)
                            pm = p_sb[:]
                            msl = mask_sb[:, ti, bs]
                        else:
                            c0 = 128 * r
                            s_ps = psR.tile([128, 512], F32, tag="ps")
                            nc.tensor.matmul(
                                s_ps[:, : 512 - c0],
                                k_sb[:, ti * 128 : (ti + 1) * 128],
                                q_sb[:, hh, j * 512 + c0 : (j + 1) * 512],
                                start=True,
                                stop=True,
                                skip_group_check=True,
                            )
                            p_sb = pfull.tile([128, 512], BF)
                            nc.scalar.activation(
                                p_sb[:, c0:], s_ps[:, : 512 - c0], AF.Exp
                            )
                            # only the 128-col diagonal sub-block is
                            # partially masked; past it cmask is all-ones
                            nc.vector.tensor_mul(
                                p_sb[:, c0 : c0 + 128],
                                p_sb[:, c0 : c0 + 128],
                                cmask_sb[:, 0:128],
                            )
                            pm = p_sb[:, c0:]
                            msl = mask_sb[:, ti, j * 8 + 2 * r : (j + 1) * 8]
                        mb = AP(
                            tensor=msl.tensor,
                            offset=msl.offset,
                            ap=list(msl.ap) + [[0, BLK]],
                        )
                        p3 = pm.rearrange("p (b w) -> p b w", w=BLK)
                        nc.gpsimd.tensor_tensor(p3, p3, mb, op=OP.mult)
                        head_state[hh][1][ti] = (p_sb, max(r, 0) * 128)

                def emit_pv(hh, ti):
                    pv_ps, p_tiles = head_state[hh]
                    p_sb, c0 = p_tiles[ti]
                    nc.tensor.matmul(
                        pv_ps[:, c0:],
                        v_sb[:, ti, :],
                        p_sb[:, c0:],
                        start=(ti == 0),
                        stop=(ti == ntile - 1),
                        skip_group_check=True,
                    )
                    p_tiles[ti] = None

                def head_open(hh):
                    head_state[hh] = (
                        psPV.tile([D + 1, 512], F32, tag="pv", name=f"pv{hh}"),
                        [None] * ntile,
                    )

                def head_scores(hh):
                    head_open(hh)
                    for ti in range(min(PIPE, ntile)):
                        emit_score(hh, ti)
                    for ti in range(ntile):
                        if ti + PIPE < ntile:
                            emit_score(hh, ti + PIPE)
                        emit_pv(hh, ti)
                def head_norm_at(hh, jj):
                    ssl = slice(jj * 512, (jj + 1) * 512)
                    head_norm(hh, ssl)

                def norm_pre(hh):
                    pv_ps = head_state[hh][0]
                    rcb = nsb.tile([1, 512], BF)
                    with nc.allow_low_precision(reason="1/Z was already cast to bf16 for the PE broadcast"):
                        nc.vector.reciprocal(rcb[:], pv_ps[D : D + 1, :])
                    head_state[hh] = (pv_ps, rcb)

                def head_norm(hh, ssl=None):
                    if ssl is None:
                        ssl = sl
                    pv_ps, rcb = head_state[hh]
                    rb_ps = psR.tile([D, 512], F32, tag="ps")
                    nc.tensor.matmul(
                        rb_ps, ones_sb[:, :D], rcb[:], start=True, stop=True
                    )
                    rb_sb = nsb.tile([D, 512], BF)
                    nc.vector.tensor_copy(rb_sb[:], rb_ps[:])
                    nc.vector.tensor_mul(
                        attn_sb[:, hh, ssl], pv_ps[:D, :], rb_sb[:]
                    )
                    # repack for the 128-row o-proj contraction
                    for pi, (t, pd0, ps0, ln) in enumerate(qpieces[hh]):
                        eng = nc.sync if (hh + pi) % 2 == 0 else nc.gpsimd
                        eng.dma_start(
                            attnp_sb[pd0 : pd0 + ln, t, ssl],
                            attn_sb[ps0 : ps0 + ln, hh, ssl],
                        )

                if j == 0:
                    # no o-proj filler exists yet: pipeline heads 2 deep and
                    # pull chunk 1's v-projection in as extra PE filler
                    def scores_only(hh):
                        head_open(hh)
                        for ti in range(ntile):
                            emit_score(hh, ti)

                    def pvs_only(hh):
                        for ti in range(ntile):
                            emit_pv(hh, ti)

                    nxt = []
                    scores_only(0)
                    nxt.append(qkv_tile(1, 0))
                    pvs_only(0)
                    norm_pre(0)
                    scores_only(1)
                    pvs_only(1)
                    norm_pre(1)
                    head_norm(0)
                    nxt.append(qkv_tile(1, 1))
                    scores_only(2)
                    pvs_only(2)
                    norm_pre(2)
                    head_norm(1)
                    nxt.append(qkv_tile(1, 2))
                    v_tile(7)
                    scores_only(3)
                    pvs_only(3)
                    norm_pre(3)
                    head_norm(2)
                    pk1 = qkv_tile(1, 3)
                    qkv_state[1] = (nxt, pk1)
                    head_norm(3)
                else:
                    for hh in range(G):
                        head_scores(hh)
                        norm_pre(hh)
                        # o-proj matmuls of the previous chunk fill the PE
                        # while the PV->reciprocal->broadcast chain drains
                        if hh == 0 and j == 1:
                            head_norm(hh)  # oproj_tile(0) already used at the gate
                        elif hh < 3 or j == NS - 1:
                            oproj_tile(4 * (j - 1) + hh)
                            head_norm(hh)
                        else:
                            deferred.append(
                                (lambda jj, t: lambda: (oproj_tile(t), head_norm_at(3, jj)))(j, 4 * (j - 1) + 3)
                            )

            for si in range(4 * (NS - 1), 4 * NS):
                oproj_tile(si)

            if debug:
                for nm, t in [
                    ("dq", q_sb),
                    ("dk", k_sb),
                    ("dv", v_sb),
                    ("dmask", mask_sb),
                    ("dqp", qp_sb),
                    ("dkm", km_sb),
                    ("dkx", kx_sb),
                    ("dattn", attn_sb),
                    ("dattnp", attnp_sb),
                ]:
                    dd = nc.dram_tensor(
                        nm, list(t[:].shape), t[:].dtype, kind="ExternalOutput"
                    )
                    nc.sync.dma_start(dd[:], t[:])
    return nc


def _host_prep(hidden_states, cos, sin, qkv_w, o_w, gate_wq, gate_wk):
    bf = ml_dtypes.bfloat16
    X = np.asarray(hidden_states, np.float32).reshape(S, HIDDEN)
    qkv_w = np.asarray(qkv_w, np.float32)
    o_w = np.asarray(o_w, np.float32)
    cos = np.asarray(cos, np.float32)
    sin = np.asarray(sin, np.float32)

    xt = np.ascontiguousarray(X.T).astype(bf)
    scale = D**-0.5
    sgn = np.ones((D, 1), np.float32)
    sgn[: D // 2] = -1.0
    cosT = np.ascontiguousarray(cos.T)
    sinT = np.ascontiguousarray(sin.T)
    cosq = (cosT * scale).astype(bf)
    sinq = (sinT * scale * sgn).astype(bf)
    cosk = cosT.astype(bf)
    sink = (sinT * sgn).astype(bf)



    emat = np.zeros((NB, NT * 128), np.float32)  # cast to bf16 below
    for i in range(NT):
        for p in range(128):
            emat[2 * i + p // BLK, i * 128 + p] = 1.0

    bcm_full = np.where(
        np.arange(NB)[None, :] <= np.arange(NB)[:, None], 0.0, -60.0
    ).astype(np.float32)
    bcm = np.ascontiguousarray(
        bcm_full.reshape(NS, 8, NB).transpose(1, 0, 2).reshape(8, NS * NB)
    )
    eye_full = np.eye(NB, dtype=np.float32)
    eyer = np.ascontiguousarray(
        eye_full.reshape(NS, 8, NB).transpose(1, 0, 2).reshape(8, NS * NB)
    )
    # cmask[p, r*512+col] = 1 if col - p >= 128*r (k token ti*128+p causal
    # w.r.t. q token j*512+col on diagonal tiles, r = ti - 4j)
    p_i = np.arange(128)[:, None]
    col = np.arange(512)[None, :]
    cmask = (col - p_i >= 0).astype(np.float32).astype(bf)

    # k block mean is computed on-device as a SUM; fold 1/BLK into the
    # mean-pool half of gate_wk
    gwk_s = np.asarray(gate_wk, np.float32).copy()
    gwk_s[:D, :] *= 1.0 / BLK

    # gate_wq replicated per packed q row: gwqp[p, t*GH+g] = gate_wq[(128t+p)%D, g]
    gwq = np.asarray(gate_wq, np.float32)
    gwqp = np.zeros((128, 3 * GH), np.float32)
    for t in range(3):
        for p in range(128):
            r = 128 * t + p
            gwqp[p, t * GH : (t + 1) * GH] = gwq[r % D, :]

    common = dict(
        xt=xt,
        cosq=cosq,
        sinq=sinq,
        cosk=cosk,
        sink=sink,
        gwqp=gwqp,
        gwk=gwk_s,
        eye8=np.eye(8, dtype=np.float32),
        eyer=eyer,
        emat=emat.astype(bf),
        bcm=bcm,
        cmask=cmask,
    )
    maps = []
    for c in range(NCORES):
        wq = qkv_w[:, c * G * D : (c + 1) * G * D]
        wk = qkv_w[:, H * D + c * D : H * D + (c + 1) * D]
        wv = qkv_w[:, H * D + HK * D + c * D : H * D + HK * D + (c + 1) * D]
        ow = o_w[c * G * D : (c + 1) * G * D, :]  # [384, 3072]
        owp = np.ascontiguousarray(
            ow.reshape(3, 128, HIDDEN).transpose(1, 0, 2).reshape(128, 3 * HIDDEN)
        )
        maps.append(
            dict(
                common,
                wqk=np.concatenate([wq, wk], axis=1).astype(bf),
                wv=wv.astype(bf),
                owp=owp.astype(bf),
            )
        )
    return maps


def _gather(results):
    acc = np.zeros((S, HIDDEN), np.float32)
    for r in results:
        acc += np.asarray(r["out_p"]).astype(np.float32)
    return acc.reshape(1, S, HIDDEN)


def _run(inputs, trace=False):
    global _prog
    if _prog is None:
        _prog = _build()
        if not _prog.is_finalized():
            _prog.finalize()
    from concourse import bass_utils

    maps = _host_prep(**inputs)
    res = bass_utils.run_bass_kernel_spmd(
        _prog, maps, list(range(NCORES)), trace=trace
    )
    return _gather(res.results), res


def kernel(**inputs):
    out, _ = _run(inputs, trace=False)
    return out
